# revision 2
# baseline (speedup 1.0000x reference)
"""Multi-head attention (B=2, S=2048, H=1024, 16 heads x 64d) on 8 trn2 cores.

Sharding: tensor-parallel over heads (2 heads/core). Each core computes the
qkv projection for its 384 output features, attention for its 2 heads, and a
partial o_proj ([4096,1024] over its 128-feature slice). Host sums the 8
partials and adds b_o.

Device layout (per core, feature-major):
  QT/KT [128, 4096]  rows = head_local*64 + d, cols = b*2048 + s  (fp32r)
  S^T orientation for scores ([k, q]) so softmax-sum over k falls out of the
  PV matmul via a ones-column appended to V; exp runs on ScalarE from PSUM;
  normalization = reciprocal of the sums row + ones-broadcast matmul + mul.
Matmuls run in fp16 (softmax attention is insensitive to score/prob rounding;
measured end-to-end rel err ~8e-4); the softmax normalizer chain stays fp32r.
Emission software-pipelines S/PV around the exp and drains a fine-grained
filler queue (qkv batch 1 / V transposes / o_proj) inside exp shadows.
"""
import sys

sys.path.insert(0, "/opt/trn_rl_repo")
import numpy as np

NHEADS = 16
HEAD_DIM = 64
HIDDEN = 1024
QKV = NHEADS * HEAD_DIM  # 1024
SCALING = HEAD_DIM ** -0.5
B = 2
S = 2048
T = B * S  # 4096
NCORES = 8
HPC = NHEADS // NCORES  # 2 heads per core
FEAT = HPC * HEAD_DIM  # 128
CHUNK = 512
NCHUNK = S // CHUNK  # 4 per batch
KSLABS = HIDDEN // 128  # 8
SSLABS = S // 128  # 16
D1 = HEAD_DIM + 1  # 65

_CACHE = {}
LAST_RESULT = None  # BassKernelResults of the most recent kernel() call


def _split_waits(nc, keep=1):
    """Hoist excess per-instruction sem waits into standalone EventSemaphore
    instructions (walrus codegen has small per-opcode wait budgets)."""
    import bass_rust
    import concourse.mybir as mybir

    n_new = 0
    for f in nc.m.functions:
        for blk in f.blocks:
            out = []
            changed = False
            for inst in blk.instructions:
                si = inst.sync_info
                waits = list(si.on_wait) if si is not None else []
                if len(waits) > keep:
                    excess = waits[: len(waits) - keep]
                    kept = waits[len(waits) - keep:]
                    for w in excess:
                        out.append(mybir.InstEventSemaphore(
                            name=f"{inst.name}-esw{n_new}",
                            engine=inst.engine,
                            sync_info=bass_rust.SyncInfo(on_wait=[w], on_update=[]),
                        ))
                        n_new += 1
                    inst.sync_info = bass_rust.SyncInfo(
                        on_wait=kept, on_update=list(si.on_update))
                    changed = True
                out.append(inst)
            if changed:
                blk.instructions = out
    return n_new


def _build(reps=1):
    import concourse.bass as bass
    import concourse.mybir as mybir
    import concourse.tile as tile
    from concourse.masks import make_identity

    f32 = mybir.dt.float32
    f32r = mybir.dt.float32r
    f16 = mybir.dt.float16
    Exp = mybir.ActivationFunctionType.Exp

    nc = bass.Bass()
    xT = nc.dram_tensor("xT", [HIDDEN, T], f16, kind="ExternalInput")
    wqkvT = nc.dram_tensor("wqkvT", [HIDDEN, 3 * FEAT], f16, kind="ExternalInput")
    bqkv = nc.dram_tensor("bqkv", [FEAT, 3], f32, kind="ExternalInput")
    woT = nc.dram_tensor("woT", [FEAT, HIDDEN], f16, kind="ExternalInput")
    out_d = nc.dram_tensor("out", [T, HIDDEN], f32, kind="ExternalOutput")

    with tile.TileContext(nc) as tc, nc.allow_low_precision(reason="fp32r matmuls"):
        with (
            tc.tile_pool(name="sing", bufs=1) as sing,
            tc.tile_pool(name="xp", bufs=2) as xp,
            tc.tile_pool(name="pp", bufs=3) as pp,
            tc.tile_pool(name="stg", bufs=4) as stg,
            tc.tile_pool(name="sm", bufs=2) as sm,
            tc.tile_pool(name="op", bufs=2) as op,
            tc.tile_pool(name="ps_mm", bufs=2, space="PSUM") as ps_mm,
            tc.tile_pool(name="ps_s", bufs=2, space="PSUM") as ps_s,
            tc.tile_pool(name="ps_o", bufs=2, space="PSUM") as ps_o,
        ):
            wq_sb = sing.tile([128, KSLABS, 3 * FEAT], f16, tag="wq")
            wo_sb = sing.tile([FEAT, HIDDEN], f16, tag="wo")
            bq_sb = sing.tile([FEAT, 3], f32, tag="bq")
            ident = sing.tile([128, 128], f32, tag="id")
            ones1 = sing.tile([1, HEAD_DIM], f32r, tag="on")
            QT = sing.tile([128, T], f16, tag="qt")
            KT = sing.tile([128, T], f16, tag="kt")
            VT = sing.tile([128, T], f32, tag="vt")
            OT = sing.tile([128, T], f16, tag="ot")
            Vaug = sing.tile([128, B, HPC, SSLABS, D1], f16, tag="va")

            nc.sync.dma_start(
                out=wq_sb, in_=wqkvT[:].rearrange("(s p) f -> p s f", p=128))
            nc.sync.dma_start(out=wo_sb, in_=woT[:])
            nc.sync.dma_start(out=bq_sb, in_=bqkv[:])
            make_identity(nc, ident)
            ones_f = stg.tile([1, HEAD_DIM], f32, tag="onf")
            nc.vector.memset(ones_f, 1.0)
            nc.vector.tensor_copy(ones1, ones_f)
            vst = stg.tile([128, B * HPC * SSLABS], f32, tag="vst")
            nc.vector.memset(vst, 1.0)
            nc.vector.tensor_copy(Vaug[:, :, :, :, HEAD_DIM:D1], vst)

            xT_c = xT[:].rearrange("(s p) t -> p s t", p=128)

            from collections import deque
            filler = deque()

            def drain(n=1):
                for _ in range(n):
                    if filler:
                        filler.popleft()()

            def qkv_dma(b, n):
                g = b * NCHUNK + n
                xc = xp.tile([128, KSLABS, CHUNK], f16, tag="xc", name="xc")
                nc.sync.dma_start(out=xc, in_=xT_c[:, :, g * CHUNK:(g + 1) * CHUNK])
                return xc

            def qkv_feat(b, n, m, xc):
                g = b * NCHUNK + n
                lo, hi = g * CHUNK, (g + 1) * CHUNK
                dest = (QT, KT, VT)[m]
                acc = ps_mm.tile([128, CHUNK], f32, tag="mm", name="acc")
                for s in range(KSLABS):
                    nc.tensor.matmul(
                        acc, wq_sb[:, s, m * FEAT:(m + 1) * FEAT], xc[:, s, :],
                        start=(s == 0), stop=(s == KSLABS - 1))
                nc.vector.tensor_scalar_add(
                    dest[:, lo:hi], acc, bq_sb[:, m:m + 1])

            def qkv_chunk(b, n):
                xc = qkv_dma(b, n)
                for m in range(3):
                    qkv_feat(b, n, m, xc)

            def vtrans_k(b, h, k):
                tp = ps_mm.tile([128, HEAD_DIM], f32, tag="mm", name="tp")
                nc.tensor.transpose(
                    tp,
                    VT[64 * h:64 * h + 64,
                       b * S + 128 * k: b * S + 128 * (k + 1)],
                    ident[64 * h:64 * h + 64, 64 * h:64 * h + 64])
                nc.vector.tensor_copy(Vaug[:, b, h, k, 0:HEAD_DIM], tp)

            def vtrans(b, n):
                for h in range(HPC):
                    for k in range(4 * n, 4 * n + 4):
                        vtrans_k(b, h, k)

            GRP = 2  # S-slabs per exp group (ps_s holds GRP banks x 2 bufs)

            def attn_unit(b, h, qc, mid=None):
                qlo = b * S + qc * CHUNK
                qsl = slice(qlo, qlo + CHUNK)
                o_ps = ps_o.tile([D1, CHUNK], f32, tag="o")
                ngrp = SSLABS // GRP

                def s_group(grp):
                    s_ps = ps_s.tile([128, GRP, CHUNK], f32, tag="s", name="s_ps")
                    for kk in range(GRP):
                        k = grp * GRP + kk
                        nc.tensor.matmul(
                            s_ps[:, kk, :],
                            KT[64 * h:64 * h + 64,
                               b * S + 128 * k: b * S + 128 * (k + 1)],
                            QT[64 * h:64 * h + 64, qsl],
                            start=True, stop=True)
                    pt = pp.tile([128, GRP, CHUNK], f16, tag="pt", name="pt")
                    nc.scalar.activation(out=pt, in_=s_ps, func=Exp)
                    return pt

                def pv_group(grp, pt):
                    for kk in range(GRP):
                        k = grp * GRP + kk
                        nc.tensor.matmul(
                            o_ps, Vaug[:, b, h, k, :], pt[:, kk, :],
                            start=(k == 0), stop=(k == SSLABS - 1))

                prev = s_group(0)
                for grp in range(1, ngrp):
                    if grp == ngrp // 2 and mid is not None:
                        mid()     # emit qkv chunks 2-3 before groups needing them
                    cur = s_group(grp)
                    drain()       # filler PE work runs in exp(grp-1)'s shadow
                    pv_group(grp - 1, prev)
                    prev = cur
                drain()
                pv_group(ngrp - 1, prev)
                rec = sm.tile([1, CHUNK], f32r, tag="rec")
                nc.vector.reciprocal(rec, o_ps[HEAD_DIM:D1, :])
                b_ps = ps_mm.tile([HEAD_DIM, CHUNK], f32, tag="mm")
                nc.tensor.matmul(b_ps, ones1, rec, start=True, stop=True)
                rb = sm.tile([HEAD_DIM, CHUNK], f32, tag="rb")
                nc.vector.tensor_copy(rb, b_ps)
                nc.vector.tensor_mul(
                    OT[64 * h:64 * h + 64, qsl], o_ps[0:HEAD_DIM, :], rb)

            def oproj_pair(j, jj, ost):
                # one token tile (both 512-col halves) into the staging buffer
                t = 4 * j + jj
                for nh in range(HIDDEN // CHUNK):
                    acc = ps_mm.tile([128, CHUNK], f32, tag="mm", name="acc2")
                    nc.tensor.matmul(
                        acc, OT[:, 128 * t:128 * (t + 1)],
                        wo_sb[:, nh * CHUNK:(nh + 1) * CHUNK],
                        start=True, stop=True)
                    nc.vector.tensor_copy(
                        ost[:, jj, nh * CHUNK:(nh + 1) * CHUNK], acc)

            def oproj_group_thunks(j):
                # token tiles 4j..4j+3 (tokens 512j..512j+512), one out-DMA
                box = {}

                def alloc():
                    box["ost"] = op.tile([128, 4, HIDDEN], f32, tag="ost", name="ost")

                thunks = [alloc]
                for jj in range(4):
                    thunks.append(lambda jj=jj: oproj_pair(j, jj, box["ost"]))

                def flush():
                    nc.sync.dma_start(
                        out=out_d[512 * j:512 * (j + 1), :].rearrange(
                            "(jj p) h -> p jj h", p=128),
                        in_=box["ost"])

                thunks.append(flush)
                return thunks

            # ---- emission: qkv(b0) up front, then attention with fine-grained
            # filler (qkv b1 / vtrans / oproj) drained inside exp shadows ----
            for _rep in range(reps):
                for n in range(2):
                    qkv_chunk(0, n)
                    vtrans(0, n)

                def rest_of_b0_qkv():
                    for n2 in range(2, NCHUNK):
                        qkv_chunk(0, n2)
                        vtrans(0, n2)

                for n in range(NCHUNK):
                    xc_box = {}

                    def dma_thunk(b=1, n=n, box=xc_box):
                        box["xc"] = qkv_dma(b, n)

                    filler.append(dma_thunk)
                    for m in range(3):
                        filler.append(
                            lambda n=n, m=m, box=xc_box: qkv_feat(1, n, m, box["xc"]))
                    for h in range(HPC):
                        for k in range(4 * n, 4 * n + 4):
                            filler.append(lambda h=h, k=k: vtrans_k(1, h, k))
                units_b0 = [(0, h, qc) for h in range(HPC) for qc in range(NCHUNK)]
                for i, (b, h, qc) in enumerate(units_b0):
                    attn_unit(b, h, qc, mid=rest_of_b0_qkv if i == 0 else None)
                drain(len(filler))  # anything left over
                units_b1 = [(1, h, qc) for h in range(HPC) for qc in range(NCHUNK)]
                for i, (b, h, qc) in enumerate(units_b1):
                    if h == 0:
                        filler.extend(oproj_group_thunks(qc))       # batch-0: deps done
                    attn_unit(b, h, qc)
                    if h == 1:
                        filler.extend(oproj_group_thunks(4 + qc))   # after its last dep
                drain(len(filler))

    _split_waits(nc)
    return nc


def _make_in_maps(hidden_states, w_qkv, b_qkv, w_o, b_o):
    x16 = np.ascontiguousarray(
        np.asarray(hidden_states, dtype=np.float32).reshape(T, HIDDEN).T
    ).astype(np.float16)
    w_qkv = np.asarray(w_qkv, dtype=np.float32)
    b_qkv = np.asarray(b_qkv, dtype=np.float32)
    w_o = np.asarray(w_o, dtype=np.float32)

    in_maps = []
    for c in range(NCORES):
        rq = slice(c * FEAT, (c + 1) * FEAT)
        wq = w_qkv[0:QKV][rq] * SCALING
        wk = w_qkv[QKV:2 * QKV][rq]
        wv = w_qkv[2 * QKV:3 * QKV][rq]
        bq = b_qkv[0:QKV][rq] * SCALING
        bk = b_qkv[QKV:2 * QKV][rq]
        bv = b_qkv[2 * QKV:3 * QKV][rq]
        in_maps.append({
            "xT": x16,
            "wqkvT": np.ascontiguousarray(
                np.concatenate([wq, wk, wv], axis=0).T).astype(np.float16),
            "bqkv": np.ascontiguousarray(np.stack([bq, bk, bv], axis=1)),
            "woT": np.ascontiguousarray(w_o[:, rq].T).astype(np.float16),
        })
    return in_maps


def kernel(hidden_states, w_qkv, b_qkv, w_o, b_o):
    global LAST_RESULT
    from concourse.bass_utils import run_bass_kernel_spmd
    import os

    if "nc" not in _CACHE:
        _CACHE["nc"] = _build()
    nc = _CACHE["nc"]

    b_o = np.asarray(b_o, dtype=np.float32)
    in_maps = _make_in_maps(hidden_states, w_qkv, b_qkv, w_o, b_o)

    trace = bool(os.environ.get("KERNEL_TRACE"))
    res = run_bass_kernel_spmd(nc, in_maps, list(range(NCORES)), trace=trace)
    LAST_RESULT = res

    acc = np.zeros((T, HIDDEN), dtype=np.float64)
    for c in range(NCORES):
        acc += res.results[c]["out"]
    out = (acc + b_o).astype(np.float32).reshape(B, S, HIDDEN)
    return out

